# revision 6
# baseline (speedup 1.0000x reference)
"""CfC attention kernel for 8 Trainium2 NeuronCores.

Reference computation (B=4, T=4096, C=1024, fp32):
    f = sigmoid(x @ W_f_x.T); g = tanh(x @ W_g_x.T)
    h_t = f_t * h_{t-1} + (1 - f_t) * g_t      (scan along T, h_{-1} = 0)
    out = h @ W_proj.T

Sharding: core (2b + half) handles batch b and channel half `half`
(512 of 1024 channels) for the gate matmuls and the scan; it then
computes a partial c_proj over its channel half and the host sums the
two partials per batch.  The scan itself runs as hardware
TensorTensorScan instructions (one independent recurrence per
partition) with channels on partitions and T on the free axis, so all
matmul operands/results already live in the layout the scan needs.

Sign trick: AluOpType has no reverse-subtract, so the device computes
b' = (f-1)*g and scans h' = f*h' + b' = -h; the host passes -W_proj so
the projection output comes out with the right sign.
"""

import sys

if "/opt/trn_rl_repo" not in sys.path:
    sys.path.insert(0, "/opt/trn_rl_repo")

import numpy as np

import concourse.bass as bass
import concourse.mybir as mybir
import concourse.tile as tile_mod
from concourse.bass_utils import run_bass_kernel_spmd
from concourse.tile import TileContext

F32 = mybir.dt.float32
F32R = mybir.dt.float32r
ALU = mybir.AluOpType
ACTF = mybir.ActivationFunctionType

# float32r streams the moving operand at 1 cycle/row (vs 4 for plain fp32)
# for moving dims >= 256; same 4-byte fp32 bits, PSUM accumulation unchanged.
USE_F32R = False

B, T, C = 4, 4096, 1024
P = 128          # SBUF partitions
CH = C // 2      # channels per core
TQ = 1024        # T chunk processed per phase
NQ = T // TQ     # 4 phases
NK = C // P      # 8 contraction chunks for the gate matmuls
NCT = CH // P    # 4 channel tiles per core
NTT = TQ // 512  # moving-operand tiles (N=512) per T chunk
NDH = C // 512   # output-channel halves in c_proj


def _split_multi_waits(nc, max_waits=1):
    """This walrus build rejects more than one sync wait per instruction
    (setupSyncWait: "Too many sync wait commands").  Tile emits multi-wait
    instructions freely, so hoist the excess waits onto same-engine NOPs
    placed immediately before the owning instruction — the NOPs' waits
    execute first in program order, preserving the sync semantics."""
    for fn in nc.m.functions:
        for blk in fn.blocks:
            insts = list(blk.instructions)
            out, changed = [], False
            for inst in insts:
                si = inst.sync_info
                waits = list(si.on_wait) if si is not None and si.on_wait else []
                if len(waits) > max_waits:
                    changed = True
                    for w in waits[:-max_waits]:
                        nop = mybir.InstNoOp(
                            name=nc.get_next_instruction_name(), ins=[], outs=[]
                        )
                        nop.engine = inst.engine
                        nop.sync_info = mybir.SyncInfo(on_wait=[w], on_update=[])
                        nc.register_instruction(nop)
                        out.append(nop)
                    si.on_wait = waits[-max_waits:]
                out.append(inst)
            if changed:
                blk.instructions = out


def build_program():
    nc = bass.Bass()
    xT_d = nc.dram_tensor("xT", (C, T), F32, kind="ExternalInput")    # x[b].T
    wf_d = nc.dram_tensor("wf", (C, CH), F32, kind="ExternalInput")   # W_f[ch,:].T
    wg_d = nc.dram_tensor("wg", (C, CH), F32, kind="ExternalInput")   # W_g[ch,:].T
    wp_d = nc.dram_tensor("wp", (CH, C), F32, kind="ExternalInput")   # -W_p[:,ch].T
    out_d = nc.dram_tensor("out", (T, C), F32, kind="ExternalOutput") # partial

    with TileContext(nc) as tc:
        with (
            tc.tile_pool(name="w", bufs=1) as wpool,
            tc.tile_pool(name="x", bufs=2) as xpool,
            tc.tile_pool(name="gates", bufs=2) as gpool,
            tc.tile_pool(name="h", bufs=2) as hpool,
            tc.tile_pool(name="ostage", bufs=3) as opool,
            tc.tile_pool(name="pf", bufs=3, space="PSUM") as pfpool,
            tc.tile_pool(name="pg", bufs=3, space="PSUM") as pgpool,
            tc.tile_pool(name="po", bufs=2, space="PSUM") as popool,
        ):
            # ---- persistent weights -------------------------------------
            wf_sb, wg_sb, wp_sb = [], [], []
            for k in range(NK):
                wt = wpool.tile([P, CH], F32, name=f"wf{k}", tag=f"wf{k}")
                nc.sync.dma_start(wt[:], wf_d[k * P:(k + 1) * P, :])
                wf_sb.append(wt)
            for k in range(NK):
                wt = wpool.tile([P, CH], F32, name=f"wg{k}", tag=f"wg{k}")
                nc.sync.dma_start(wt[:], wg_d[k * P:(k + 1) * P, :])
                wg_sb.append(wt)
            for cc in range(NCT):
                wt = wpool.tile([P, C], F32, name=f"wp{cc}", tag=f"wp{cc}")
                nc.sync.dma_start(wt[:], wp_d[cc * P:(cc + 1) * P, :])
                wp_sb.append(wt)

            def emit_proj(q, h_tiles):
                t0 = q * TQ
                for tt in range(TQ // P):
                    ot = opool.tile([P, C], F32, name=f"ot{q}_{tt}", tag="ot")
                    for dh in range(NDH):
                        pso = popool.tile(
                            [P, 512], F32, name=f"pso{q}_{tt}_{dh}", tag="po"
                        )
                        for cc in range(NCT):
                            nc.tensor.matmul(
                                pso[:],
                                h_tiles[cc][:, tt * P:(tt + 1) * P],
                                wp_sb[cc][:, dh * 512:(dh + 1) * 512],
                                start=(cc == 0),
                                stop=(cc == NCT - 1),
                            )
                        dst = ot[:, dh * 512:(dh + 1) * 512]
                        if dh == 0:
                            nc.scalar.copy(dst, pso[:])
                        else:
                            nc.vector.tensor_copy(dst, pso[:])
                    nc.sync.dma_start(out_d[t0 + tt * P: t0 + (tt + 1) * P, :], ot[:])

            h_prev = None
            pending_proj = None
            for q in range(NQ):
                t0 = q * TQ
                xt = []
                for k in range(NK):
                    xtile = xpool.tile(
                        [P, TQ], F32, name=f"xt{k}_{q}", tag=f"xt{k}"
                    )
                    nc.sync.dma_start(
                        xtile[:], xT_d[k * P:(k + 1) * P, t0: t0 + TQ]
                    )
                    xt.append(xtile)

                h_cur = []
                for ct in range(NCT):
                    psf = [
                        pfpool.tile([P, 512], F32, name=f"psf{q}_{ct}_{tt}", tag="pf")
                        for tt in range(NTT)
                    ]
                    psg = [
                        pgpool.tile([P, 512], F32, name=f"psg{q}_{ct}_{tt}", tag="pg")
                        for tt in range(NTT)
                    ]
                    for k in range(NK):
                        wsl = wf_sb[k][:, ct * P:(ct + 1) * P]
                        for tt in range(NTT):
                            nc.tensor.matmul(
                                psf[tt][:],
                                wsl,
                                xt[k][:, tt * 512:(tt + 1) * 512],
                                start=(k == 0),
                                stop=(k == NK - 1),
                            )
                    for k in range(NK):
                        wsl = wg_sb[k][:, ct * P:(ct + 1) * P]
                        for tt in range(NTT):
                            nc.tensor.matmul(
                                psg[tt][:],
                                wsl,
                                xt[k][:, tt * 512:(tt + 1) * 512],
                                start=(k == 0),
                                stop=(k == NK - 1),
                            )
                    f_sb = gpool.tile([P, TQ], F32, name=f"f{q}_{ct}", tag="f")
                    g_sb = gpool.tile([P, TQ], F32, name=f"g{q}_{ct}", tag="g")
                    b_sb = gpool.tile([P, TQ], F32, name=f"b{q}_{ct}", tag="b")
                    for tt in range(NTT):
                        sl = slice(tt * 512, (tt + 1) * 512)
                        nc.scalar.activation(f_sb[:, sl], psf[tt][:], ACTF.Sigmoid)
                        nc.scalar.activation(g_sb[:, sl], psg[tt][:], ACTF.Tanh)
                    # b' = (f - 1) * g  == -(1-f)*g; wp is negated to compensate
                    nc.vector.scalar_tensor_tensor(
                        b_sb[:], f_sb[:], 1.0, g_sb[:], ALU.subtract, ALU.mult
                    )
                    h_t = hpool.tile([P, TQ], F32, name=f"h{q}_{ct}", tag=f"h{ct}")
                    init = 0.0 if q == 0 else h_prev[ct][:, TQ - 1: TQ]
                    nc.vector.tensor_tensor_scan(
                        h_t[:], f_sb[:], b_sb[:], init, ALU.mult, ALU.add
                    )
                    h_cur.append(h_t)

                # software pipeline: c_proj of phase q-1 is emitted after the
                # gate matmuls of phase q so PE never waits on the DVE scan
                if pending_proj is not None:
                    emit_proj(*pending_proj)
                pending_proj = (q, h_cur)
                h_prev = h_cur

            emit_proj(*pending_proj)

    _split_multi_waits(nc)
    return nc


_NC_CACHE = None


def _get_nc():
    global _NC_CACHE
    if _NC_CACHE is None:
        _NC_CACHE = build_program()
    return _NC_CACHE


def make_in_maps(x, W_f_x, W_g_x, W_proj):
    x = np.asarray(x, dtype=np.float32)
    W_f_x = np.asarray(W_f_x, dtype=np.float32)
    W_g_x = np.asarray(W_g_x, dtype=np.float32)
    W_proj = np.asarray(W_proj, dtype=np.float32)
    in_maps = []
    for b in range(B):
        xTb = np.ascontiguousarray(x[b].T)  # (C, T)
        for half in range(2):
            ch = slice(half * CH, (half + 1) * CH)
            in_maps.append(
                {
                    "xT": xTb,
                    "wf": np.ascontiguousarray(W_f_x[ch, :].T),
                    "wg": np.ascontiguousarray(W_g_x[ch, :].T),
                    "wp": np.ascontiguousarray(-W_proj[:, ch].T),
                }
            )
    return in_maps


def kernel(x, W_f_x, W_g_x, W_proj):
    nc = _get_nc()
    in_maps = make_in_maps(x, W_f_x, W_g_x, W_proj)
    res = run_bass_kernel_spmd(nc, in_maps, core_ids=list(range(2 * B)))
    out = np.empty((B, T, C), dtype=np.float32)
    for b in range(B):
        out[b] = res.results[2 * b]["out"] + res.results[2 * b + 1]["out"]
    return out


if __name__ == "__main__":
    rng = np.random.default_rng(0)
    x = rng.standard_normal((B, T, C), dtype=np.float32)
    s = 1.0 / np.sqrt(C)
    wf = rng.standard_normal((C, C), dtype=np.float32) * s
    wg = rng.standard_normal((C, C), dtype=np.float32) * s
    wp = rng.standard_normal((C, C), dtype=np.float32) * s
    out = kernel(x=x, W_f_x=wf, W_g_x=wg, W_proj=wp)
    print("out", out.shape, out.dtype, float(np.abs(out).mean()))


# revision 12
# speedup vs baseline: 31.3991x; 31.3991x over previous
"""CfC attention kernel for 8 Trainium2 NeuronCores.

Reference computation (B=4, T=4096, C=1024, fp32):
    f = sigmoid(x @ W_f_x.T); g = tanh(x @ W_g_x.T)
    h_t = f_t * h_{t-1} + (1 - f_t) * g_t      (scan along T, h_{-1} = 0)
    out = h @ W_proj.T

Sharding: core (2b + half) handles batch b and channel half `half`
(512 of 1024 channels) for the gate matmuls and the scan; it then
computes a partial c_proj over its channel half and the host sums the
two partials per batch.  The scan itself runs as hardware
TensorTensorScan instructions (one independent recurrence per
partition) with channels on partitions and T on the free axis, so all
matmul operands/results already live in the layout the scan needs.

Sign trick: AluOpType has no reverse-subtract, so the device computes
b' = (f-1)*g and scans h' = f*h' + b' = -h; the host passes -W_proj so
the projection output comes out with the right sign.

Matmul operands are float32r (fp32 bits, TF32-like rounded multiply):
1 cycle/row on the PE vs 4 for plain fp32, measured HW rel err ~2e-4.
"""

import sys

if "/opt/trn_rl_repo" not in sys.path:
    sys.path.insert(0, "/opt/trn_rl_repo")

import numpy as np

import concourse.bass as bass
import concourse.mybir as mybir
from concourse.bass_utils import run_bass_kernel_spmd
from concourse.tile import TileContext

F32 = mybir.dt.float32
F32R = mybir.dt.float32r
ALU = mybir.AluOpType
ACTF = mybir.ActivationFunctionType

# float32r streams the moving operand at 1 cycle/row (vs 4 for plain fp32)
# for moving dims >= 256; same 4-byte fp32 bits, PSUM accumulation unchanged.
USE_F32R = True

B, T, C = 4, 4096, 1024
P = 128          # SBUF partitions
CH = C // 2      # channels per core
TQ = 1024        # T chunk processed per phase
NQ = T // TQ     # 4 phases
NK = C // P      # 8 contraction chunks for the gate matmuls
NCT = CH // P    # 4 channel tiles per core
NTT = TQ // 512  # moving-operand tiles (N=512) per T chunk
NDH = C // 512   # output-channel halves in c_proj


def _split_multi_waits(nc, max_waits=1):
    """This walrus build rejects more than one sync wait per instruction
    (setupSyncWait: "Too many sync wait commands").  Tile emits multi-wait
    instructions freely, so hoist the excess waits onto same-engine NOPs
    placed immediately before the owning instruction — the NOPs' waits
    execute first in program order, preserving the sync semantics."""
    for fn in nc.m.functions:
        for blk in fn.blocks:
            insts = list(blk.instructions)
            out, changed = [], False
            for inst in insts:
                si = inst.sync_info
                waits = list(si.on_wait) if si is not None and si.on_wait else []
                if len(waits) > max_waits:
                    changed = True
                    for w in waits[:-max_waits]:
                        nop = mybir.InstNoOp(
                            name=nc.get_next_instruction_name(), ins=[], outs=[]
                        )
                        nop.engine = inst.engine
                        nop.sync_info = mybir.SyncInfo(on_wait=[w], on_update=[])
                        nc.register_instruction(nop)
                        out.append(nop)
                    si.on_wait = waits[-max_waits:]
                out.append(inst)
            if changed:
                blk.instructions = out


def build_program(repeat=1):
    # float32r operands must be *produced* as float32r (BIR verifier), so the
    # matmul-feeding DRAM tensors and SBUF tiles are declared f32r end-to-end.
    MMDT = F32R if USE_F32R else F32
    nc = bass.Bass()
    xT_d = nc.dram_tensor("xT", (C, T), MMDT, kind="ExternalInput")    # x[b].T
    wf_d = nc.dram_tensor("wf", (C, CH), MMDT, kind="ExternalInput")   # W_f[ch,:].T
    wg_d = nc.dram_tensor("wg", (C, CH), MMDT, kind="ExternalInput")   # W_g[ch,:].T
    wp_d = nc.dram_tensor("wp", (CH, C), MMDT, kind="ExternalInput")   # -W_p[:,ch].T
    out_d = nc.dram_tensor("out", (T, C), F32, kind="ExternalOutput")  # partial

    def mm(ap):
        return ap

    with TileContext(nc) as tc:
        with (
            tc.tile_pool(name="w", bufs=1) as wpool,
            tc.tile_pool(name="x", bufs=2) as xpool,
            tc.tile_pool(name="gates", bufs=2) as gpool,
            tc.tile_pool(name="h", bufs=2) as hpool,
            tc.tile_pool(name="ostage", bufs=3) as opool,
            tc.tile_pool(name="pf", bufs=3, space="PSUM") as pfpool,
            tc.tile_pool(name="pg", bufs=3, space="PSUM") as pgpool,
            tc.tile_pool(name="po", bufs=2, space="PSUM") as popool,
        ):
            # ---- persistent weights -------------------------------------
          # (re-loaded per benchmark repetition; tags serialize reps safely)
          for _rep in range(repeat):
            # DMA emission order matters at startup: interleave so the
            # first gate matmul's operands (wf0, wg0, xt0 of quarter 0) land
            # first; wp is not needed until the first c_proj (~60us in).
            wf_sb, wg_sb, wp_sb, xt0 = [], [], [], []
            for k in range(NK):
                wt = wpool.tile([P, CH], MMDT, name=f"wf{k}", tag=f"wf{k}")
                nc.sync.dma_start(wt[:], wf_d[k * P:(k + 1) * P, :])
                wf_sb.append(wt)
                wt = wpool.tile([P, CH], MMDT, name=f"wg{k}", tag=f"wg{k}")
                nc.sync.dma_start(wt[:], wg_d[k * P:(k + 1) * P, :])
                wg_sb.append(wt)
                xtile = xpool.tile([P, TQ], MMDT, name=f"xt{k}_0", tag=f"xt{k}")
                nc.sync.dma_start(xtile[:], xT_d[k * P:(k + 1) * P, 0:TQ])
                xt0.append(xtile)
            for cc in range(NCT):
                wt = wpool.tile([P, C], MMDT, name=f"wp{cc}", tag=f"wp{cc}")
                nc.sync.dma_start(wt[:], wp_d[cc * P:(cc + 1) * P, :])
                wp_sb.append(wt)

            def emit_proj(q, h_tiles):
                t0 = q * TQ
                for tt in range(TQ // P):
                    ot = opool.tile([P, C], F32, name=f"ot{q}_{tt}", tag="ot")
                    for dh in range(NDH):
                        pso = popool.tile(
                            [P, 512], F32, name=f"pso{q}_{tt}_{dh}", tag="po"
                        )
                        for cc in range(NCT):
                            nc.tensor.matmul(
                                pso[:],
                                mm(h_tiles[cc][:, tt * P:(tt + 1) * P]),
                                mm(wp_sb[cc][:, dh * 512:(dh + 1) * 512]),
                                start=(cc == 0),
                                stop=(cc == NCT - 1),
                            )
                        dst = ot[:, dh * 512:(dh + 1) * 512]
                        if dh == 0:
                            nc.scalar.copy(dst, pso[:])
                        else:
                            nc.vector.tensor_copy(dst, pso[:])
                    nc.sync.dma_start(out_d[t0 + tt * P: t0 + (tt + 1) * P, :], ot[:])

            h_prev = None
            pending_proj = None
            for q in range(NQ):
                t0 = q * TQ
                if q == 0:
                    xt = xt0
                else:
                    xt = []
                    for k in range(NK):
                        xtile = xpool.tile(
                            [P, TQ], MMDT, name=f"xt{k}_{q}", tag=f"xt{k}"
                        )
                        nc.sync.dma_start(
                            xtile[:], xT_d[k * P:(k + 1) * P, t0: t0 + TQ]
                        )
                        xt.append(xtile)

                h_cur = []
                for ct in range(NCT):
                    psf = [
                        pfpool.tile([P, 512], F32, name=f"psf{q}_{ct}_{tt}", tag="pf")
                        for tt in range(NTT)
                    ]
                    psg = [
                        pgpool.tile([P, 512], F32, name=f"psg{q}_{ct}_{tt}", tag="pg")
                        for tt in range(NTT)
                    ]
                    for k in range(NK):
                        wsl = wf_sb[k][:, ct * P:(ct + 1) * P]
                        for tt in range(NTT):
                            nc.tensor.matmul(
                                psf[tt][:],
                                mm(wsl),
                                mm(xt[k][:, tt * 512:(tt + 1) * 512]),
                                start=(k == 0),
                                stop=(k == NK - 1),
                            )
                    for k in range(NK):
                        wsl = wg_sb[k][:, ct * P:(ct + 1) * P]
                        for tt in range(NTT):
                            nc.tensor.matmul(
                                psg[tt][:],
                                mm(wsl),
                                mm(xt[k][:, tt * 512:(tt + 1) * 512]),
                                start=(k == 0),
                                stop=(k == NK - 1),
                            )
                    f_sb = gpool.tile([P, TQ], F32, name=f"f{q}_{ct}", tag="f")
                    g_sb = gpool.tile([P, TQ], F32, name=f"g{q}_{ct}", tag="g")
                    b_sb = gpool.tile([P, TQ], F32, name=f"b{q}_{ct}", tag="b")
                    for tt in range(NTT):
                        sl = slice(tt * 512, (tt + 1) * 512)
                        nc.scalar.activation(f_sb[:, sl], psf[tt][:], ACTF.Sigmoid)
                        nc.scalar.activation(g_sb[:, sl], psg[tt][:], ACTF.Tanh)
                    # b' = (f - 1) * g  == -(1-f)*g; wp is negated to compensate
                    nc.vector.scalar_tensor_tensor(
                        b_sb[:], f_sb[:], 1.0, g_sb[:], ALU.subtract, ALU.mult
                    )
                    h_t = hpool.tile([P, TQ], MMDT, name=f"h{q}_{ct}", tag=f"h{ct}")
                    init = 0.0 if q == 0 else h_prev[ct][:, TQ - 1: TQ]
                    nc.vector.tensor_tensor_scan(
                        h_t[:], f_sb[:], b_sb[:], init, ALU.mult, ALU.add
                    )
                    h_cur.append(h_t)

                # software pipeline: c_proj of phase q-1 is emitted after the
                # gate matmuls of phase q so PE never waits on the DVE scan
                if pending_proj is not None:
                    emit_proj(*pending_proj)
                pending_proj = (q, h_cur)
                h_prev = h_cur

            emit_proj(*pending_proj)

    _split_multi_waits(nc)
    return nc


_NC_CACHE = None


def _get_nc():
    global _NC_CACHE
    if _NC_CACHE is None:
        _NC_CACHE = build_program()
    return _NC_CACHE


def make_in_maps(x, W_f_x, W_g_x, W_proj):
    x = np.asarray(x, dtype=np.float32)
    W_f_x = np.asarray(W_f_x, dtype=np.float32)
    W_g_x = np.asarray(W_g_x, dtype=np.float32)
    W_proj = np.asarray(W_proj, dtype=np.float32)
    in_maps = []
    for b in range(B):
        xTb = np.ascontiguousarray(x[b].T)  # (C, T)
        for half in range(2):
            ch = slice(half * CH, (half + 1) * CH)
            in_maps.append(
                {
                    "xT": xTb,
                    "wf": np.ascontiguousarray(W_f_x[ch, :].T),
                    "wg": np.ascontiguousarray(W_g_x[ch, :].T),
                    "wp": np.ascontiguousarray(-W_proj[:, ch].T),
                }
            )
    return in_maps


def kernel(x, W_f_x, W_g_x, W_proj):
    nc = _get_nc()
    in_maps = make_in_maps(x, W_f_x, W_g_x, W_proj)
    res = run_bass_kernel_spmd(nc, in_maps, core_ids=list(range(2 * B)))
    out = np.empty((B, T, C), dtype=np.float32)
    for b in range(B):
        out[b] = res.results[2 * b]["out"] + res.results[2 * b + 1]["out"]
    return out


if __name__ == "__main__":
    rng = np.random.default_rng(0)
    x = rng.standard_normal((B, T, C), dtype=np.float32)
    s = 1.0 / np.sqrt(C)
    wf = rng.standard_normal((C, C), dtype=np.float32) * s
    wg = rng.standard_normal((C, C), dtype=np.float32) * s
    wp = rng.standard_normal((C, C), dtype=np.float32) * s
    out = kernel(x=x, W_f_x=wf, W_g_x=wg, W_proj=wp)
    print("out", out.shape, out.dtype, float(np.abs(out).mean()))


# revision 15
# speedup vs baseline: 33.3878x; 1.0633x over previous
"""CfC attention kernel for 8 Trainium2 NeuronCores.

Reference computation (B=4, T=4096, C=1024, fp32):
    f = sigmoid(x @ W_f_x.T); g = tanh(x @ W_g_x.T)
    h_t = f_t * h_{t-1} + (1 - f_t) * g_t      (scan along T, h_{-1} = 0)
    out = h @ W_proj.T

Sharding: core (2b + half) handles batch b and channel half `half`
(512 of 1024 channels) for the gate matmuls and the scan; it then
computes a partial c_proj over its channel half and the host sums the
two partials per batch.  The scan itself runs as hardware
TensorTensorScan instructions (one independent recurrence per
partition) with channels on partitions and T on the free axis, so all
matmul operands/results already live in the layout the scan needs.

Sign trick: AluOpType has no reverse-subtract, so the device computes
b' = (f-1)*g and scans h' = f*h' + b' = -h; the host passes -W_proj so
the projection output comes out with the right sign.

Matmul operands are float32r (fp32 bits, TF32-like rounded multiply):
1 cycle/row on the PE vs 4 for plain fp32, measured HW rel err ~2e-4.
"""

import sys

if "/opt/trn_rl_repo" not in sys.path:
    sys.path.insert(0, "/opt/trn_rl_repo")

import numpy as np

import concourse.bass as bass
import concourse.mybir as mybir
from concourse.bass_utils import run_bass_kernel_spmd
from concourse.tile import TileContext

F32 = mybir.dt.float32
F32R = mybir.dt.float32r
BF16 = mybir.dt.bfloat16
ALU = mybir.AluOpType
ACTF = mybir.ActivationFunctionType

# float32r streams the moving operand at 1 cycle/row (vs 4 for plain fp32)
# for moving dims >= 256; same 4-byte fp32 bits, PSUM accumulation unchanged.
USE_F32R = True
# bf16 gate-matmul inputs (x, W_f, W_g) were A/B-tested: only ~9us/iter
# faster (the kernel is PE-streaming-bound, not input-DMA-bound) at 12x
# the numerical error (2.8e-3 vs 2.3e-4) — not worth it.  Keep f32r.
USE_BF16_GATES = False

B, T, C = 4, 4096, 1024
P = 128          # SBUF partitions
CH = C // 2      # channels per core
TQ = 1024        # T chunk processed per phase
NQ = T // TQ     # 4 phases
NK = C // P      # 8 contraction chunks for the gate matmuls
NCT = CH // P    # 4 channel tiles per core
NTT = TQ // 512  # moving-operand tiles (N=512) per T chunk
NDH = C // 512   # output-channel halves in c_proj


def _split_multi_waits(nc, max_waits=1):
    """This walrus build rejects more than one sync wait per instruction
    (setupSyncWait: "Too many sync wait commands").  Tile emits multi-wait
    instructions freely, so hoist the excess waits onto same-engine NOPs
    placed immediately before the owning instruction — the NOPs' waits
    execute first in program order, preserving the sync semantics."""
    for fn in nc.m.functions:
        for blk in fn.blocks:
            insts = list(blk.instructions)
            out, changed = [], False
            for inst in insts:
                si = inst.sync_info
                waits = list(si.on_wait) if si is not None and si.on_wait else []
                if len(waits) > max_waits:
                    changed = True
                    for w in waits[:-max_waits]:
                        nop = mybir.InstNoOp(
                            name=nc.get_next_instruction_name(), ins=[], outs=[]
                        )
                        nop.engine = inst.engine
                        nop.sync_info = mybir.SyncInfo(on_wait=[w], on_update=[])
                        nc.register_instruction(nop)
                        out.append(nop)
                    si.on_wait = waits[-max_waits:]
                out.append(inst)
            if changed:
                blk.instructions = out


def build_program(repeat=1):
    # float32r operands must be *produced* as float32r (BIR verifier), so the
    # matmul-feeding DRAM tensors and SBUF tiles are declared f32r end-to-end.
    MMDT = F32R if USE_F32R else F32
    GDT = BF16 if USE_BF16_GATES else MMDT  # gate-matmul input dtype
    nc = bass.Bass()
    xT_d = nc.dram_tensor("xT", (C, T), GDT, kind="ExternalInput")     # x[b].T
    wf_d = nc.dram_tensor("wf", (C, CH), GDT, kind="ExternalInput")    # W_f[ch,:].T
    wg_d = nc.dram_tensor("wg", (C, CH), GDT, kind="ExternalInput")    # W_g[ch,:].T
    wp_d = nc.dram_tensor("wp", (CH, C), MMDT, kind="ExternalInput")   # -W_p[:,ch].T
    out_d = nc.dram_tensor("out", (T, C), F32, kind="ExternalOutput")  # partial

    def mm(ap):
        return ap

    with TileContext(nc) as tc:
        with (
            tc.tile_pool(name="w", bufs=1) as wpool,
            tc.tile_pool(name="x", bufs=2) as xpool,
            tc.tile_pool(name="gates", bufs=2) as gpool,
            tc.tile_pool(name="h", bufs=2) as hpool,
            tc.tile_pool(name="ostage", bufs=3) as opool,
            tc.tile_pool(name="pf", bufs=3, space="PSUM") as pfpool,
            tc.tile_pool(name="pg", bufs=3, space="PSUM") as pgpool,
            tc.tile_pool(name="po", bufs=2, space="PSUM") as popool,
        ):
            # ---- persistent weights -------------------------------------
          # (re-loaded per benchmark repetition; tags serialize reps safely)
          for _rep in range(repeat):
            # DMA emission order matters at startup: interleave so the
            # first gate matmul's operands (wf0, wg0, xt0 of quarter 0) land
            # first; wp is not needed until the first c_proj (~60us in).
            wf_sb, wg_sb, wp_sb, xt0 = [], [], [], []
            for k in range(NK):
                wt = wpool.tile([P, CH], GDT, name=f"wf{k}", tag=f"wf{k}")
                nc.sync.dma_start(wt[:], wf_d[k * P:(k + 1) * P, :])
                wf_sb.append(wt)
                wt = wpool.tile([P, CH], GDT, name=f"wg{k}", tag=f"wg{k}")
                nc.sync.dma_start(wt[:], wg_d[k * P:(k + 1) * P, :])
                wg_sb.append(wt)
                xtile = xpool.tile([P, TQ], GDT, name=f"xt{k}_0", tag=f"xt{k}")
                nc.sync.dma_start(xtile[:], xT_d[k * P:(k + 1) * P, 0:TQ])
                xt0.append(xtile)
            for cc in range(NCT):
                wt = wpool.tile([P, C], MMDT, name=f"wp{cc}", tag=f"wp{cc}")
                nc.sync.dma_start(wt[:], wp_d[cc * P:(cc + 1) * P, :])
                wp_sb.append(wt)

            def emit_proj(q, h_tiles):
                t0 = q * TQ
                for tt in range(TQ // P):
                    ot = opool.tile([P, C], F32, name=f"ot{q}_{tt}", tag="ot")
                    for dh in range(NDH):
                        pso = popool.tile(
                            [P, 512], F32, name=f"pso{q}_{tt}_{dh}", tag="po"
                        )
                        for cc in range(NCT):
                            nc.tensor.matmul(
                                pso[:],
                                mm(h_tiles[cc][:, tt * P:(tt + 1) * P]),
                                mm(wp_sb[cc][:, dh * 512:(dh + 1) * 512]),
                                start=(cc == 0),
                                stop=(cc == NCT - 1),
                            )
                        dst = ot[:, dh * 512:(dh + 1) * 512]
                        if dh == 0:
                            nc.scalar.copy(dst, pso[:])
                        else:
                            nc.vector.tensor_copy(dst, pso[:])
                    nc.sync.dma_start(out_d[t0 + tt * P: t0 + (tt + 1) * P, :], ot[:])

            h_prev = None
            pending_proj = None
            for q in range(NQ):
                t0 = q * TQ
                if q == 0:
                    xt = xt0
                else:
                    xt = []
                    for k in range(NK):
                        xtile = xpool.tile(
                            [P, TQ], GDT, name=f"xt{k}_{q}", tag=f"xt{k}"
                        )
                        nc.sync.dma_start(
                            xtile[:], xT_d[k * P:(k + 1) * P, t0: t0 + TQ]
                        )
                        xt.append(xtile)

                h_cur = []
                for ct in range(NCT):
                    psf = [
                        pfpool.tile([P, 512], F32, name=f"psf{q}_{ct}_{tt}", tag="pf")
                        for tt in range(NTT)
                    ]
                    psg = [
                        pgpool.tile([P, 512], F32, name=f"psg{q}_{ct}_{tt}", tag="pg")
                        for tt in range(NTT)
                    ]
                    for k in range(NK):
                        wsl = wf_sb[k][:, ct * P:(ct + 1) * P]
                        for tt in range(NTT):
                            nc.tensor.matmul(
                                psf[tt][:],
                                mm(wsl),
                                mm(xt[k][:, tt * 512:(tt + 1) * 512]),
                                start=(k == 0),
                                stop=(k == NK - 1),
                            )
                    for k in range(NK):
                        wsl = wg_sb[k][:, ct * P:(ct + 1) * P]
                        for tt in range(NTT):
                            nc.tensor.matmul(
                                psg[tt][:],
                                mm(wsl),
                                mm(xt[k][:, tt * 512:(tt + 1) * 512]),
                                start=(k == 0),
                                stop=(k == NK - 1),
                            )
                    f_sb = gpool.tile([P, TQ], F32, name=f"f{q}_{ct}", tag="f")
                    g_sb = gpool.tile([P, TQ], F32, name=f"g{q}_{ct}", tag="g")
                    b_sb = gpool.tile([P, TQ], F32, name=f"b{q}_{ct}", tag="b")
                    for tt in range(NTT):
                        sl = slice(tt * 512, (tt + 1) * 512)
                        nc.scalar.activation(f_sb[:, sl], psf[tt][:], ACTF.Sigmoid)
                        nc.scalar.activation(g_sb[:, sl], psg[tt][:], ACTF.Tanh)
                    # b' = (f - 1) * g  == -(1-f)*g; wp is negated to compensate
                    nc.vector.scalar_tensor_tensor(
                        b_sb[:], f_sb[:], 1.0, g_sb[:], ALU.subtract, ALU.mult
                    )
                    h_t = hpool.tile([P, TQ], MMDT, name=f"h{q}_{ct}", tag=f"h{ct}")
                    init = 0.0 if q == 0 else h_prev[ct][:, TQ - 1: TQ]
                    nc.vector.tensor_tensor_scan(
                        h_t[:], f_sb[:], b_sb[:], init, ALU.mult, ALU.add
                    )
                    h_cur.append(h_t)

                # software pipeline: c_proj of phase q-1 is emitted after the
                # gate matmuls of phase q so PE never waits on the DVE scan
                if pending_proj is not None:
                    emit_proj(*pending_proj)
                pending_proj = (q, h_cur)
                h_prev = h_cur

            emit_proj(*pending_proj)

    _split_multi_waits(nc)
    return nc


_NC_CACHE = None


def _get_nc():
    global _NC_CACHE
    if _NC_CACHE is None:
        _NC_CACHE = build_program()
    return _NC_CACHE


def make_in_maps(x, W_f_x, W_g_x, W_proj):
    x = np.asarray(x, dtype=np.float32)
    W_f_x = np.asarray(W_f_x, dtype=np.float32)
    W_g_x = np.asarray(W_g_x, dtype=np.float32)
    W_proj = np.asarray(W_proj, dtype=np.float32)
    if USE_BF16_GATES:
        import ml_dtypes

        gdt = ml_dtypes.bfloat16
    else:
        gdt = np.float32
    in_maps = []
    for b in range(B):
        xTb = np.ascontiguousarray(x[b].T.astype(gdt))  # (C, T)
        for half in range(2):
            ch = slice(half * CH, (half + 1) * CH)
            in_maps.append(
                {
                    "xT": xTb,
                    "wf": np.ascontiguousarray(W_f_x[ch, :].T.astype(gdt)),
                    "wg": np.ascontiguousarray(W_g_x[ch, :].T.astype(gdt)),
                    "wp": np.ascontiguousarray(-W_proj[:, ch].T),
                }
            )
    return in_maps


def kernel(x, W_f_x, W_g_x, W_proj):
    nc = _get_nc()
    in_maps = make_in_maps(x, W_f_x, W_g_x, W_proj)
    res = run_bass_kernel_spmd(nc, in_maps, core_ids=list(range(2 * B)))
    out = np.empty((B, T, C), dtype=np.float32)
    for b in range(B):
        out[b] = res.results[2 * b]["out"] + res.results[2 * b + 1]["out"]
    return out


if __name__ == "__main__":
    rng = np.random.default_rng(0)
    x = rng.standard_normal((B, T, C), dtype=np.float32)
    s = 1.0 / np.sqrt(C)
    wf = rng.standard_normal((C, C), dtype=np.float32) * s
    wg = rng.standard_normal((C, C), dtype=np.float32) * s
    wp = rng.standard_normal((C, C), dtype=np.float32) * s
    out = kernel(x=x, W_f_x=wf, W_g_x=wg, W_proj=wp)
    print("out", out.shape, out.dtype, float(np.abs(out).mean()))
